# revision 37
# baseline (speedup 1.0000x reference)
"""Trainium2 Bass kernel for nn_DistillingLayer: per-channel shared-weight
Conv1d(k=3, stride=2, pad=1) + ELU + MaxPool1d(k=3, stride=2, pad=1) over
x:(16, 4096, 512) f32 -> out:(16, 1024, 512) f32.

Strategy
--------
- Data-parallel over batch: 8 cores x 2 batches each. No communication.
- Layout: L lives in the SBUF *free* dimension. Each partition owns S
  consecutive L-rows (x D=512 channels) plus a 3-row halo loaded with
  overlap from HBM, so the whole conv+pool dataflow stays per-partition
  local (DVE lanes cannot cross partitions).
- The input is zero-padded by 3 L-rows on the host. This makes every tile
  a uniform full-128-partition DMA (SWDGE descriptor fan-out degenerates
  onto 1-2 of the 16 SDMA engines for partition counts other than 128)
  and supplies the conv's left zero-padding for free.
- ELU is monotonic, so maxpool commutes with it: pool the pre-activation
  conv outputs, then apply ELU once on the pooled result (half the rows).
  The conv bias is folded into the conv's first tap pass (ScalarE Copy
  applies scale and bias).
- Conv c[i] = w0*x[2i-1] + w1*x[2i] + w2*x[2i+1] + bias is one ScalarE
  activation (first tap + bias) and two VectorE scalar_tensor_tensor
  accumulate passes over strided views; out[j] = max(c[2j-1],c[2j],c[2j+1])
  is two VectorE tensor_tensor max passes.
- ELU(v) = max(v, exp(min(v,0)) - 1): min/exp via two ScalarE activations
  (Relu with scale=-1, then Exp with scale=-1), final fused
  (e-1) max v on VectorE scalar_tensor_tensor.
- Output DMAs are emitted one tile late on the gpsimd queue so their
  wait-for-compute never gates the next tile's input DMA trigger.
- Weights/bias are baked as immediates (kernel() receives concrete values);
  the compiled module is cached per (w, b) value.

Toolchain workaround (see inline comment): a BIR post-pass splits
multi-wait instructions — this walrus build allows one sync wait per
instruction.
"""

import json as _json
import os
import sys

import numpy as np

for _p in ("/opt/trn_rl_repo", "/root/.axon_site/_ro/trn_rl_repo"):
    if os.path.isdir(_p) and _p not in sys.path:
        sys.path.append(_p)

import concourse.bass as bass
import concourse.bass2jax as bass2jax
import concourse.bass_utils as bass_utils
import concourse.mybir as mybir
from concourse.bass_utils import run_bass_kernel_spmd
from concourse.tile import TileContext

# ---------------------------------------------------------------------------
# REQUIRED workaround: this container's walrus build rejects instructions
# carrying more than one sync wait ("Too many sync wait commands" in
# setupSyncWait). Tile's scheduler freely attaches several waits to one
# instruction, so post-process the BIR JSON before compile: hoist all but the
# last wait onto same-engine NoOps inserted just before the instruction
# (per-engine program order makes sequential waits equivalent to a
# multi-wait).
# ---------------------------------------------------------------------------

_orig_compile_bir_kernel = bass_utils.compile_bir_kernel


def _split_multi_waits(bir_json: bytes) -> bytes:
    j = _json.loads(bir_json)
    ctr = 0
    changed = False
    for fn in j["functions"]:
        for bb in fn["blocks"]:
            out = []
            for ins in bb["instructions"]:
                si = ins.get("sync_info")
                waits = (si.get("on_wait") or []) if si else []
                if len(waits) > 1:
                    changed = True
                    for w in waits[:-1]:
                        ctr += 1
                        out.append(
                            {
                                "debug": ins.get("debug", 0),
                                "engine": ins["engine"],
                                "ins": [],
                                "outs": [],
                                "name": f"waitsplit-{ctr}",
                                "opcode": "NoOp",
                                "text_hint": "waitsplit",
                                "sync_info": {"on_update": [], "on_wait": [w]},
                            }
                        )
                    si["on_wait"] = [waits[-1]]
                out.append(ins)
            bb["instructions"] = out
    if not changed:
        return bir_json
    return _json.dumps(j).encode()


def _patched_compile_bir_kernel(bir_json, tmpdir, neff_name="file.neff"):
    return _orig_compile_bir_kernel(_split_multi_waits(bir_json), tmpdir, neff_name)


bass_utils.compile_bir_kernel = _patched_compile_bir_kernel
bass2jax.compile_bir_kernel = _patched_compile_bir_kernel

# ---------------------------------------------------------------------------

N_CORES = 8
B, L, D = 16, 4096, 512
BPC = B // N_CORES  # batches per core
LC = L // 2         # conv output length
LP = LC // 2        # pool output length

F32 = mybir.dt.float32
ALU = mybir.AluOpType
AF = mybir.ActivationFunctionType

_cache: dict = {}

# Exposed for test harnesses: the BassKernelResults of the last run.
LAST_RESULT = None


def _build(w0: float, w1: float, w2: float, bias: float) -> bass.Bass:
    nc = bass.Bass()
    # x is host-padded with 3 zero rows at the front of L: padded row r
    # holds true row r-3 (see module docstring).
    x = nc.dram_tensor("x", [BPC, L + 3, D], F32, kind="ExternalInput")
    y = nc.dram_tensor("y", [BPC, LP, D], F32, kind="ExternalOutput")

    xrow = D              # elements per L-row
    xbat = (L + 3) * D    # elements per (padded) input batch
    ybat = LP * D

    # Tile schedule per batch: (row_base, S_t); rows covered = 128 * S_t.
    # Per-batch schedules: small tiles at the global start (fast pipeline
    # fill) and end (short drain tail), big in the middle.
    sched = None

    with TileContext(nc) as tc:
        with (
            tc.tile_pool(name="xp", bufs=3) as xp,
            tc.tile_pool(name="yp", bufs=3) as yp,
            tc.tile_pool(name="pp", bufs=2) as pp,
            tc.tile_pool(name="rp", bufs=2) as rp,
        ):
            # Output DMAs are emitted one tile LATE on the gpsimd queue so
            # their wait-for-compute is already satisfied when the trigger
            # executes and never gates the next tile's input DMA trigger
            # (engine DMA queues execute waits in program order).
            pending_out = None
            sched0 = [(0, 8), (1024, 8), (2048, 16)]
            sched1 = [(0, 16), (2048, 8), (3072, 8)]
            tiles = [(b, rb, st)
                     for b in range(BPC)
                     for rb, st in (sched0 if b == 0 else sched1)]
            for b, row_base, St in tiles:
                Q = St // 2 + 1   # conv rows per partition (incl. 1 halo row)
                Jt = St // 4      # pool-output rows per partition
                # Partition p holds padded rows [row_base + p*St,
                # row_base + p*St + St + 3) = true rows [R0-3, R0+St) with
                # R0 = row_base + p*St: 3 halo rows, then its own S rows.
                X = xp.tile([128, (St + 3) * D], F32)
                nc.gpsimd.dma_start(
                    out=X[:, :],
                    in_=bass.AP(
                        x,
                        b * xbat + row_base * xrow,
                        [[St * xrow, 128], [1, (St + 3) * xrow]],
                    ),
                )
                if pending_out is not None:
                    nc.gpsimd.dma_start(out=pending_out[0], in_=pending_out[1])
                    pending_out = None

                Xv = X[:, :].rearrange("p (r d) -> p r d", d=D)
                # conv row q (local) = c[2*O0 - 1 + q], O0 = R0/4; its taps
                # are x rows (local) 2q, 2q+1, 2q+2
                ya = Xv[:, 0 : St + 1 : 2, :]
                yb = Xv[:, 1 : St + 2 : 2, :]
                yc = Xv[:, 2 : St + 3 : 2, :]

                Y = yp.tile([128, Q * D], F32)
                y3 = Y[:, :].rearrange("p (q d) -> p q d", d=D)

                # conv (bias folded in): c = w0*ya + w1*yb + w2*yc + bias
                nc.scalar.activation(y3, ya, AF.Copy, bias=bias, scale=w0)
                nc.vector.scalar_tensor_tensor(
                    y3, yb, w1, y3, op0=ALU.mult, op1=ALU.add
                )
                nc.vector.scalar_tensor_tensor(
                    y3, yc, w2, y3, op0=ALU.mult, op1=ALU.add
                )
                if row_base == 0:
                    # left pool pad: c[-1] = -inf (partition 0 only)
                    nc.vector.memset(Y[0:1, 0:D], float("-inf"))

                # maxpool (pre-activation; ELU is monotonic):
                # out[jl] = max(Y[2jl], Y[2jl+1], Y[2jl+2])
                P = pp.tile([128, Jt * D], F32)
                p3 = P[:, :].rearrange("p (j d) -> p j d", d=D)
                nc.vector.tensor_tensor(
                    p3,
                    y3[:, 0 : 2 * Jt - 1 : 2, :],
                    y3[:, 1 : 2 * Jt : 2, :],
                    op=ALU.max,
                )
                nc.vector.tensor_tensor(
                    p3, p3, y3[:, 2 : 2 * Jt + 1 : 2, :], op=ALU.max
                )

                # ELU(v) = max(v, exp(min(v,0)) - 1)
                R = rp.tile([128, Jt * D], F32)
                nc.scalar.activation(R[:, :], P[:, :], AF.Relu, scale=-1.0)
                nc.scalar.activation(R[:, :], R[:, :], AF.Exp, scale=-1.0)
                nc.vector.scalar_tensor_tensor(
                    R[:, :], R[:, :], -1.0, P[:, :], op0=ALU.add, op1=ALU.max
                )

                pending_out = (
                    bass.AP(
                        y,
                        b * ybat + (row_base // 4) * xrow,
                        [[Jt * D, 128], [1, Jt * D]],
                    ),
                    R[:, :],
                )
            nc.gpsimd.dma_start(out=pending_out[0], in_=pending_out[1])
    return nc


def kernel(x: np.ndarray, w: np.ndarray, b: np.ndarray) -> np.ndarray:
    global LAST_RESULT
    w = np.asarray(w, dtype=np.float32)
    bb = np.asarray(b, dtype=np.float32)
    key = (float(w[0]), float(w[1]), float(w[2]), float(bb[0]))
    if key not in _cache:
        _cache[key] = _build(*key)
    nc = _cache[key]

    x = np.asarray(x, dtype=np.float32)
    assert x.shape == (B, L, D), x.shape
    xpad = np.zeros((B, L + 3, D), dtype=np.float32)
    xpad[:, 3:, :] = x
    in_maps = [
        {"x": np.ascontiguousarray(xpad[c * BPC : (c + 1) * BPC])}
        for c in range(N_CORES)
    ]
    res = run_bass_kernel_spmd(nc, in_maps, core_ids=list(range(N_CORES)))
    LAST_RESULT = res
    return np.concatenate([r["y"] for r in res.results], axis=0)


# revision 38
# speedup vs baseline: 1.0244x; 1.0244x over previous
"""Trainium2 Bass kernel for nn_DistillingLayer: per-channel shared-weight
Conv1d(k=3, stride=2, pad=1) + ELU + MaxPool1d(k=3, stride=2, pad=1) over
x:(16, 4096, 512) f32 -> out:(16, 1024, 512) f32.

Strategy
--------
- Data-parallel over batch: 8 cores x 2 batches each. No communication.
- Layout: L lives in the SBUF *free* dimension. Each partition owns S
  consecutive L-rows (x D=512 channels) plus a 3-row halo loaded with
  overlap from HBM, so the whole conv+pool dataflow stays per-partition
  local (DVE lanes cannot cross partitions).
- The input is zero-padded by 3 L-rows on the host. This makes every tile
  a uniform full-128-partition DMA (SWDGE descriptor fan-out degenerates
  onto 1-2 of the 16 SDMA engines for partition counts other than 128)
  and supplies the conv's left zero-padding for free.
- ELU is monotonic, so maxpool commutes with it: pool the pre-activation
  conv outputs, then apply ELU once on the pooled result (half the rows).
  The conv bias is folded into the conv's first tap pass (ScalarE Copy
  applies scale and bias).
- Conv c[i] = w0*x[2i-1] + w1*x[2i] + w2*x[2i+1] + bias is one ScalarE
  activation (first tap + bias) and two VectorE scalar_tensor_tensor
  accumulate passes over strided views; out[j] = max(c[2j-1],c[2j],c[2j+1])
  is two VectorE tensor_tensor max passes.
- ELU(v) = max(v, exp(min(v,0)) - 1): min/exp via two ScalarE activations
  (Relu with scale=-1, then Exp with scale=-1), final fused
  (e-1) max v on VectorE scalar_tensor_tensor.
- Output DMAs are emitted one tile late on the gpsimd queue so their
  wait-for-compute never gates the next tile's input DMA trigger.
- Weights/bias are baked as immediates (kernel() receives concrete values);
  the compiled module is cached per (w, b) value.

Toolchain workaround (see inline comment): a BIR post-pass splits
multi-wait instructions — this walrus build allows one sync wait per
instruction.
"""

import json as _json
import os
import sys

import numpy as np

for _p in ("/opt/trn_rl_repo", "/root/.axon_site/_ro/trn_rl_repo"):
    if os.path.isdir(_p) and _p not in sys.path:
        sys.path.append(_p)

import concourse.bass as bass
import concourse.bass2jax as bass2jax
import concourse.bass_utils as bass_utils
import concourse.mybir as mybir
from concourse.bass_utils import run_bass_kernel_spmd
from concourse.tile import TileContext

# ---------------------------------------------------------------------------
# REQUIRED workaround: this container's walrus build rejects instructions
# carrying more than one sync wait ("Too many sync wait commands" in
# setupSyncWait). Tile's scheduler freely attaches several waits to one
# instruction, so post-process the BIR JSON before compile: hoist all but the
# last wait onto same-engine NoOps inserted just before the instruction
# (per-engine program order makes sequential waits equivalent to a
# multi-wait).
# ---------------------------------------------------------------------------

_orig_compile_bir_kernel = bass_utils.compile_bir_kernel


def _split_multi_waits(bir_json: bytes) -> bytes:
    j = _json.loads(bir_json)
    ctr = 0
    changed = False
    for fn in j["functions"]:
        for bb in fn["blocks"]:
            out = []
            for ins in bb["instructions"]:
                si = ins.get("sync_info")
                waits = (si.get("on_wait") or []) if si else []
                if len(waits) > 1:
                    changed = True
                    for w in waits[:-1]:
                        ctr += 1
                        out.append(
                            {
                                "debug": ins.get("debug", 0),
                                "engine": ins["engine"],
                                "ins": [],
                                "outs": [],
                                "name": f"waitsplit-{ctr}",
                                "opcode": "NoOp",
                                "text_hint": "waitsplit",
                                "sync_info": {"on_update": [], "on_wait": [w]},
                            }
                        )
                    si["on_wait"] = [waits[-1]]
                out.append(ins)
            bb["instructions"] = out
    if not changed:
        return bir_json
    return _json.dumps(j).encode()


def _patched_compile_bir_kernel(bir_json, tmpdir, neff_name="file.neff"):
    return _orig_compile_bir_kernel(_split_multi_waits(bir_json), tmpdir, neff_name)


bass_utils.compile_bir_kernel = _patched_compile_bir_kernel
bass2jax.compile_bir_kernel = _patched_compile_bir_kernel

# ---------------------------------------------------------------------------

N_CORES = 8
B, L, D = 16, 4096, 512
BPC = B // N_CORES  # batches per core
LC = L // 2         # conv output length
LP = LC // 2        # pool output length

F32 = mybir.dt.float32
ALU = mybir.AluOpType
AF = mybir.ActivationFunctionType

_cache: dict = {}

# Exposed for test harnesses: the BassKernelResults of the last run.
LAST_RESULT = None


def _build(w0: float, w1: float, w2: float, bias: float) -> bass.Bass:
    nc = bass.Bass()
    # x is host-padded with 3 zero rows at the front of L: padded row r
    # holds true row r-3 (see module docstring).
    x = nc.dram_tensor("x", [BPC, L + 3, D], F32, kind="ExternalInput")
    y = nc.dram_tensor("y", [BPC, LP, D], F32, kind="ExternalOutput")

    xrow = D              # elements per L-row
    xbat = (L + 3) * D    # elements per (padded) input batch
    ybat = LP * D

    # Tile schedule per batch: (row_base, S_t); rows covered = 128 * S_t.
    sched = [(0, 16), (2048, 16)]

    with TileContext(nc) as tc:
        with (
            tc.tile_pool(name="xp", bufs=3) as xp,
            tc.tile_pool(name="yp", bufs=2) as yp,
            tc.tile_pool(name="pp", bufs=2) as pp,
            tc.tile_pool(name="rp", bufs=3) as rp,
        ):
            # Output DMAs are emitted one tile LATE on the gpsimd queue so
            # their wait-for-compute is already satisfied when the trigger
            # executes and never gates the next tile's input DMA trigger
            # (engine DMA queues execute waits in program order).
            pending_out = None
            tiles = [(b, rb, st) for b in range(BPC) for rb, st in sched]
            for b, row_base, St in tiles:
                Q = St // 2 + 1   # conv rows per partition (incl. 1 halo row)
                Jt = St // 4      # pool-output rows per partition
                # Partition p holds padded rows [row_base + p*St,
                # row_base + p*St + St + 3) = true rows [R0-3, R0+St) with
                # R0 = row_base + p*St: 3 halo rows, then its own S rows.
                X = xp.tile([128, (St + 3) * D], F32)
                nc.gpsimd.dma_start(
                    out=X[:, :],
                    in_=bass.AP(
                        x,
                        b * xbat + row_base * xrow,
                        [[St * xrow, 128], [1, (St + 3) * xrow]],
                    ),
                )
                if pending_out is not None:
                    nc.gpsimd.dma_start(out=pending_out[0], in_=pending_out[1])
                    pending_out = None

                Xv = X[:, :].rearrange("p (r d) -> p r d", d=D)
                # conv row q (local) = c[2*O0 - 1 + q], O0 = R0/4; its taps
                # are x rows (local) 2q, 2q+1, 2q+2
                ya = Xv[:, 0 : St + 1 : 2, :]
                yb = Xv[:, 1 : St + 2 : 2, :]
                yc = Xv[:, 2 : St + 3 : 2, :]

                Y = yp.tile([128, Q * D], F32)
                y3 = Y[:, :].rearrange("p (q d) -> p q d", d=D)

                # conv (bias folded in): c = w0*ya + w1*yb + w2*yc + bias
                nc.scalar.activation(y3, ya, AF.Copy, bias=bias, scale=w0)
                nc.vector.scalar_tensor_tensor(
                    y3, yb, w1, y3, op0=ALU.mult, op1=ALU.add
                )
                nc.vector.scalar_tensor_tensor(
                    y3, yc, w2, y3, op0=ALU.mult, op1=ALU.add
                )
                if row_base == 0:
                    # left pool pad: c[-1] = -inf (partition 0 only)
                    nc.vector.memset(Y[0:1, 0:D], float("-inf"))

                # maxpool (pre-activation; ELU is monotonic):
                # out[jl] = max(Y[2jl], Y[2jl+1], Y[2jl+2])
                P = pp.tile([128, Jt * D], F32)
                p3 = P[:, :].rearrange("p (j d) -> p j d", d=D)
                nc.vector.tensor_tensor(
                    p3,
                    y3[:, 0 : 2 * Jt - 1 : 2, :],
                    y3[:, 1 : 2 * Jt : 2, :],
                    op=ALU.max,
                )
                nc.vector.tensor_tensor(
                    p3, p3, y3[:, 2 : 2 * Jt + 1 : 2, :], op=ALU.max
                )

                # ELU(v) = max(v, exp(min(v,0)) - 1)
                R = rp.tile([128, Jt * D], F32)
                nc.scalar.activation(R[:, :], P[:, :], AF.Relu, scale=-1.0)
                nc.scalar.activation(R[:, :], R[:, :], AF.Exp, scale=-1.0)
                nc.vector.scalar_tensor_tensor(
                    R[:, :], R[:, :], -1.0, P[:, :], op0=ALU.add, op1=ALU.max
                )

                pending_out = (
                    bass.AP(
                        y,
                        b * ybat + (row_base // 4) * xrow,
                        [[Jt * D, 128], [1, Jt * D]],
                    ),
                    R[:, :],
                )
            nc.gpsimd.dma_start(out=pending_out[0], in_=pending_out[1])
    return nc


def kernel(x: np.ndarray, w: np.ndarray, b: np.ndarray) -> np.ndarray:
    global LAST_RESULT
    w = np.asarray(w, dtype=np.float32)
    bb = np.asarray(b, dtype=np.float32)
    key = (float(w[0]), float(w[1]), float(w[2]), float(bb[0]))
    if key not in _cache:
        _cache[key] = _build(*key)
    nc = _cache[key]

    x = np.asarray(x, dtype=np.float32)
    assert x.shape == (B, L, D), x.shape
    xpad = np.zeros((B, L + 3, D), dtype=np.float32)
    xpad[:, 3:, :] = x
    in_maps = [
        {"x": np.ascontiguousarray(xpad[c * BPC : (c + 1) * BPC])}
        for c in range(N_CORES)
    ]
    res = run_bass_kernel_spmd(nc, in_maps, core_ids=list(range(N_CORES)))
    LAST_RESULT = res
    return np.concatenate([r["y"] for r in res.results], axis=0)


# revision 40
# speedup vs baseline: 1.0343x; 1.0097x over previous
"""Trainium2 Bass kernel for nn_DistillingLayer: per-channel shared-weight
Conv1d(k=3, stride=2, pad=1) + ELU + MaxPool1d(k=3, stride=2, pad=1) over
x:(16, 4096, 512) f32 -> out:(16, 1024, 512) f32.

Strategy
--------
- Data-parallel over batch: 8 cores x 2 batches each. No communication.
- Layout: L lives in the SBUF *free* dimension. Each partition owns S
  consecutive L-rows (x D=512 channels) plus a 3-row halo loaded with
  overlap from HBM, so the whole conv+pool dataflow stays per-partition
  local (DVE lanes cannot cross partitions).
- The input is zero-padded by 3 L-rows on the host. This makes every tile
  a uniform full-128-partition DMA (SWDGE descriptor fan-out degenerates
  onto 1-2 of the 16 SDMA engines for partition counts other than 128)
  and supplies the conv's left zero-padding for free.
- ELU is monotonic, so maxpool commutes with it: pool the pre-activation
  conv outputs, then apply ELU once on the pooled result (half the rows).
  The conv bias is folded into the conv's first tap pass (ScalarE Copy
  applies scale and bias).
- Conv c[i] = w0*x[2i-1] + w1*x[2i] + w2*x[2i+1] + bias is one ScalarE
  activation (first tap + bias) and two VectorE scalar_tensor_tensor
  accumulate passes over strided views; out[j] = max(c[2j-1],c[2j],c[2j+1])
  is two VectorE tensor_tensor max passes.
- ELU(v) = max(v, exp(min(v,0)) - 1): min/exp via two ScalarE activations
  (Relu with scale=-1, then Exp with scale=-1), final fused
  (e-1) max v on VectorE scalar_tensor_tensor.
- Output DMAs are emitted one tile late on the gpsimd queue so their
  wait-for-compute never gates the next tile's input DMA trigger.
- Weights/bias are baked as immediates (kernel() receives concrete values);
  the compiled module is cached per (w, b) value.

Toolchain workaround (see inline comment): a BIR post-pass splits
multi-wait instructions — this walrus build allows one sync wait per
instruction.
"""

import json as _json
import os
import sys

import numpy as np

for _p in ("/opt/trn_rl_repo", "/root/.axon_site/_ro/trn_rl_repo"):
    if os.path.isdir(_p) and _p not in sys.path:
        sys.path.append(_p)

import concourse.bass as bass
import concourse.bass2jax as bass2jax
import concourse.bass_utils as bass_utils
import concourse.mybir as mybir
from concourse.bass_utils import run_bass_kernel_spmd
from concourse.tile import TileContext

# ---------------------------------------------------------------------------
# REQUIRED workaround: this container's walrus build rejects instructions
# carrying more than one sync wait ("Too many sync wait commands" in
# setupSyncWait). Tile's scheduler freely attaches several waits to one
# instruction, so post-process the BIR JSON before compile: hoist all but the
# last wait onto same-engine NoOps inserted just before the instruction
# (per-engine program order makes sequential waits equivalent to a
# multi-wait).
# ---------------------------------------------------------------------------

_orig_compile_bir_kernel = bass_utils.compile_bir_kernel


def _split_multi_waits(bir_json: bytes) -> bytes:
    j = _json.loads(bir_json)
    ctr = 0
    changed = False
    for fn in j["functions"]:
        for bb in fn["blocks"]:
            out = []
            for ins in bb["instructions"]:
                si = ins.get("sync_info")
                waits = (si.get("on_wait") or []) if si else []
                if len(waits) > 1:
                    changed = True
                    for w in waits[:-1]:
                        ctr += 1
                        out.append(
                            {
                                "debug": ins.get("debug", 0),
                                "engine": ins["engine"],
                                "ins": [],
                                "outs": [],
                                "name": f"waitsplit-{ctr}",
                                "opcode": "NoOp",
                                "text_hint": "waitsplit",
                                "sync_info": {"on_update": [], "on_wait": [w]},
                            }
                        )
                    si["on_wait"] = [waits[-1]]
                out.append(ins)
            bb["instructions"] = out
    if not changed:
        return bir_json
    return _json.dumps(j).encode()


def _patched_compile_bir_kernel(bir_json, tmpdir, neff_name="file.neff"):
    return _orig_compile_bir_kernel(_split_multi_waits(bir_json), tmpdir, neff_name)


bass_utils.compile_bir_kernel = _patched_compile_bir_kernel
bass2jax.compile_bir_kernel = _patched_compile_bir_kernel

# ---------------------------------------------------------------------------

N_CORES = 8
B, L, D = 16, 4096, 512
BPC = B // N_CORES  # batches per core
LC = L // 2         # conv output length
LP = LC // 2        # pool output length

F32 = mybir.dt.float32
ALU = mybir.AluOpType
AF = mybir.ActivationFunctionType

_cache: dict = {}

# Exposed for test harnesses: the BassKernelResults of the last run.
LAST_RESULT = None


def _build(w0: float, w1: float, w2: float, bias: float) -> bass.Bass:
    nc = bass.Bass()
    # x is host-padded with 3 zero rows at the front of L: padded row r
    # holds true row r-3 (see module docstring).
    x = nc.dram_tensor("x", [BPC, L + 3, D], F32, kind="ExternalInput")
    y = nc.dram_tensor("y", [BPC, LP, D], F32, kind="ExternalOutput")

    xrow = D              # elements per L-row
    xbat = (L + 3) * D    # elements per (padded) input batch
    ybat = LP * D

    # Tile schedule per batch: (row_base, S_t); rows covered = 128 * S_t.
    sched = [(0, 16), (2048, 16)]

    with TileContext(nc) as tc:
        with (
            tc.tile_pool(name="xp", bufs=3) as xp,
            tc.tile_pool(name="yp", bufs=3) as yp,
            tc.tile_pool(name="pp", bufs=2) as pp,
            tc.tile_pool(name="rp", bufs=2) as rp,
        ):
            # Output DMAs are emitted one tile LATE on the gpsimd queue so
            # their wait-for-compute is already satisfied when the trigger
            # executes and never gates the next tile's input DMA trigger
            # (engine DMA queues execute waits in program order).
            pending_out = None
            tiles = [(b, rb, st) for b in range(BPC) for rb, st in sched]
            for b, row_base, St in tiles:
                Q = St // 2 + 1   # conv rows per partition (incl. 1 halo row)
                Jt = St // 4      # pool-output rows per partition
                # Partition p holds padded rows [row_base + p*St,
                # row_base + p*St + St + 3) = true rows [R0-3, R0+St) with
                # R0 = row_base + p*St: 3 halo rows, then its own S rows.
                X = xp.tile([128, (St + 3) * D], F32)
                nc.gpsimd.dma_start(
                    out=X[:, :],
                    in_=bass.AP(
                        x,
                        b * xbat + row_base * xrow,
                        [[St * xrow, 128], [1, (St + 3) * xrow]],
                    ),
                )
                if pending_out is not None:
                    nc.gpsimd.dma_start(out=pending_out[0], in_=pending_out[1])
                    pending_out = None

                Xv = X[:, :].rearrange("p (r d) -> p r d", d=D)
                # conv row q (local) = c[2*O0 - 1 + q], O0 = R0/4; its taps
                # are x rows (local) 2q, 2q+1, 2q+2
                ya = Xv[:, 0 : St + 1 : 2, :]
                yb = Xv[:, 1 : St + 2 : 2, :]
                yc = Xv[:, 2 : St + 3 : 2, :]

                Y = yp.tile([128, Q * D], F32)
                y3 = Y[:, :].rearrange("p (q d) -> p q d", d=D)

                # conv (bias folded in): c = w0*ya + w1*yb + w2*yc + bias
                nc.scalar.activation(y3, ya, AF.Copy, bias=bias, scale=w0)
                nc.vector.scalar_tensor_tensor(
                    y3, yb, w1, y3, op0=ALU.mult, op1=ALU.add
                )
                nc.vector.scalar_tensor_tensor(
                    y3, yc, w2, y3, op0=ALU.mult, op1=ALU.add
                )
                if row_base == 0:
                    # left pool pad: c[-1] = -inf (partition 0 only)
                    nc.vector.memset(Y[0:1, 0:D], float("-inf"))

                # maxpool (pre-activation; ELU is monotonic):
                # out[jl] = max(Y[2jl], Y[2jl+1], Y[2jl+2])
                P = pp.tile([128, Jt * D], F32)
                p3 = P[:, :].rearrange("p (j d) -> p j d", d=D)
                nc.vector.tensor_tensor(
                    p3,
                    y3[:, 0 : 2 * Jt - 1 : 2, :],
                    y3[:, 1 : 2 * Jt : 2, :],
                    op=ALU.max,
                )
                nc.vector.tensor_tensor(
                    p3, p3, y3[:, 2 : 2 * Jt + 1 : 2, :], op=ALU.max
                )

                # ELU(v) = max(v, exp(min(v,0)) - 1)
                R = rp.tile([128, Jt * D], F32)
                nc.scalar.activation(R[:, :], P[:, :], AF.Relu, scale=-1.0)
                nc.scalar.activation(R[:, :], R[:, :], AF.Exp, scale=-1.0)
                nc.vector.scalar_tensor_tensor(
                    R[:, :], R[:, :], -1.0, P[:, :], op0=ALU.add, op1=ALU.max
                )

                pending_out = (
                    bass.AP(
                        y,
                        b * ybat + (row_base // 4) * xrow,
                        [[Jt * D, 128], [1, Jt * D]],
                    ),
                    R[:, :],
                )
            nc.gpsimd.dma_start(out=pending_out[0], in_=pending_out[1])
    return nc


def kernel(x: np.ndarray, w: np.ndarray, b: np.ndarray) -> np.ndarray:
    global LAST_RESULT
    w = np.asarray(w, dtype=np.float32)
    bb = np.asarray(b, dtype=np.float32)
    key = (float(w[0]), float(w[1]), float(w[2]), float(bb[0]))
    if key not in _cache:
        _cache[key] = _build(*key)
    nc = _cache[key]

    x = np.asarray(x, dtype=np.float32)
    assert x.shape == (B, L, D), x.shape
    xpad = np.zeros((B, L + 3, D), dtype=np.float32)
    xpad[:, 3:, :] = x
    in_maps = [
        {"x": np.ascontiguousarray(xpad[c * BPC : (c + 1) * BPC])}
        for c in range(N_CORES)
    ]
    res = run_bass_kernel_spmd(nc, in_maps, core_ids=list(range(N_CORES)))
    LAST_RESULT = res
    return np.concatenate([r["y"] for r in res.results], axis=0)
